# revision 4
# baseline (speedup 1.0000x reference)
"""Binarized 4-layer MLP (8192x784 -> 6144 -> 6144 -> 6144 -> 10, log_softmax)
on 8 Trainium2 NeuronCores, data-parallel over the batch.

Per-core dataflow (batch slice of 1024, feature-major activations [feat, batch]):
  fc1: x @ sign(w1).T as a 2-term fp16 hi/lo split of x, with the two terms
       stacked along the contraction dim (1568 rows -> 13 k-tiles). fp16
       upconverts losslessly to the PE's e10m11 internal format and the
       weights are exactly +-1, so this reproduces fp32 accuracy.
  fc2/fc3: one level of Strassen over the 1024x6144 @ 6144x6144 binary
       matmul: 7 half-size products instead of 8 (12.5% fewer PE cycles).
       Activation-side combos are {-2,0,2} (exact in fp8, built on the DVE);
       weight-side combos are precomputed on the host. Products run in fp8e4
       DoubleRow; all values are small integers so fp32 PSUM stays bit-exact.
  fc4: fused into the fc3 f-loop, single fp16 pass.
  log_softmax: max-free, transpose-first: per 128-batch chunk, PE-transpose
       the 10 logits, then Exp/reduce/Ln/subtract run on [128,10] tiles.

Scheduling (v2): every Strassen S-tile (10 for fc2, 10 for fc3) is
chunk-built on the DVE inside the loop that produces its inputs, so the
fc1->fc2 and fc2->fc3 phase boundaries have no serial whole-tile builds.
To make the 20 S-tiles fit, raw h1/h2 are never materialized: only the
quarters the direct Strassen products read (A11-left, A22-right) persist,
plus a 2-row staging buffer for combo inputs. fc2-output-side tiles live on
the right SBUF stack so their lifetime (mid-fc2 -> fc3-end) can overlap the
left stack's phase pools. First-f weight tiles for fc2/fc3 are prefetched /
JIT-loaded so the PE never waits on weight DMA at a phase switch. ~150
dummy matmuls at t=0 hold the PE busy so the HAM clock-gate is warm when
the first real matmul issues.
"""

import numpy as np
import ml_dtypes

import concourse.bass as bass
import concourse.mybir as mybir
from concourse import bacc
from concourse.tile import TileContext
from concourse.bass_utils import run_bass_kernel_spmd
from concourse.masks import make_identity

dt = mybir.dt

CORES = 8
B = 8192
BC = B // CORES          # 1024 batch rows per core
DIN = 784
KT1 = 13                 # fc1 contraction tiles: 2*784 = 1568 padded to 1664
K1P = KT1 * 128
DH = 6144
MT = DH // 128           # 48 feature tiles
KBS = 12                 # DoubleRow blocks per 3072 Strassen half
NF = 24                  # 128-wide output chunks per 3072 block-column
DOUT = 10
NH = BC // 512           # 2 moving halves of 512
MQ = 24                  # fc1 m-groups (w1 streamed per 2 m-tiles)
MPQ = MT // MQ
NWARM = 150              # PE pre-warm matmuls (FD=128) to beat the HAM gate

BF16 = ml_dtypes.bfloat16
FP8 = mybir.dt.np(dt.float8e4)

last_exec_time_ns = None

# Strassen product order: direct-operand chains (M4) first; combines fire as
# soon as their inputs exist. M3/M4 read the persisted quarters directly,
# the rest read S-combos.
ORDER = [4, 7, 5, 1, 2, 3, 6]


def _build_program():
    nc = bacc.Bacc("TRN2", target_bir_lowering=False, debug=False,
                   num_devices=CORES)

    xt = nc.dram_tensor("xt", [128, KT1, BC], dt.float16,
                        kind="ExternalInput").ap()
    w1t = nc.dram_tensor("w1t", [MQ, 128, KT1, MPQ * 128], dt.float16,
                         kind="ExternalInput").ap()
    w2s = nc.dram_tensor("w2s", [NF, 128, 7, KBS, 2, 128], dt.float8e4,
                         kind="ExternalInput").ap()
    w3s = nc.dram_tensor("w3s", [NF, 128, 7, KBS, 2, 128], dt.float8e4,
                         kind="ExternalInput").ap()
    w4p = nc.dram_tensor("w4p", [128, MT, DOUT], dt.float16,
                         kind="ExternalInput").ap()
    b1p = nc.dram_tensor("b1p", [128, MT], dt.float32, kind="ExternalInput").ap()
    b2p = nc.dram_tensor("b2p", [128, MT], dt.float32, kind="ExternalInput").ap()
    b3p = nc.dram_tensor("b3p", [128, MT], dt.float32, kind="ExternalInput").ap()
    b4p = nc.dram_tensor("b4p", [DOUT, 1], dt.float32, kind="ExternalInput").ap()
    out = nc.dram_tensor("out", [BC, DOUT], dt.float32, kind="ExternalOutput").ap()

    DR = mybir.MatmulPerfMode.DoubleRow
    AF = mybir.ActivationFunctionType
    ADD = mybir.AluOpType.add
    SUB = mybir.AluOpType.subtract
    MULT = mybir.AluOpType.mult

    def tt(out_, a, b_, op):
        nc.vector.scalar_tensor_tensor(out_, a, 1.0, b_, MULT, op)

    with TileContext(nc) as tc:
        # ---------------- persistent pools (left stack) ----------------
        # w2pool/c2pool sit at the BOTTOM of the stack (below the fc1-side
        # tiles) and stay alive through fc3, so the fc3 pools allocate in
        # the region freed by s2pool/h1-quarters, whose release only waits
        # on PE readers (not the slow DVE combine drain).
        cpool = tc.alloc_tile_pool(name="consts", bufs=1)
        w2pool = tc.alloc_tile_pool(name="w2pool", bufs=6)
        c2pool = tc.alloc_tile_pool(name="c2pool", bufs=1)
        # fc1-output side, live fc1+fc2: A11|A21 rows (0..23) split L/R,
        # A22 right quarter (rows 24..47), all fp8 sign outputs.
        h1lp = tc.alloc_tile_pool(name="h1lp", bufs=1)
        a22r1p = tc.alloc_tile_pool(name="a22r1p", bufs=1)
        s2pool = tc.alloc_tile_pool(name="s2pool", bufs=1)
        # PE warm-up PSUM target on the right PSUM stack (never conflicts
        # with the phase psum pools).
        warmps = tc.alloc_tile_pool(name="warmps", bufs=1, space="PSUM",
                                    side="right")

        h1L = h1lp.tile([128, NF, 512], dt.float8e4)
        a22r1 = a22r1p.tile([128, NF, 512], dt.float8e4)
        S2 = {}
        for px in range(2):
            for i in (1, 2, 5, 6, 7):
                S2[(i, px)] = s2pool.tile([128, NF, 256], dt.float8e4,
                                          tag=f"s2_{i}_{px}",
                                          name=f"s2_{i}_{px}")

        # consts
        b1_sb = cpool.tile([128, MT], dt.float32)
        nc.sync.dma_start(out=b1_sb[:], in_=b1p[:])
        b2_sb = cpool.tile([128, MT], dt.float32)
        nc.gpsimd.dma_start(out=b2_sb[:], in_=b2p[:])
        b3_sb = cpool.tile([128, MT], dt.float32)
        nc.sync.dma_start(out=b3_sb[:], in_=b3p[:])
        b4_sb = cpool.tile([DOUT, 1], dt.float32)
        nc.gpsimd.dma_start(out=b4_sb[:], in_=b4p[:])
        w4_sb = cpool.tile([128, MT, DOUT], dt.float16)
        nc.sync.dma_start(out=w4_sb[:], in_=w4p[:])
        ident = cpool.tile([33, 33], dt.float32)
        make_identity(nc, ident[:])
        ones_sb = cpool.tile([DOUT, 1], dt.float16)
        nc.vector.memset(ones_sb[:], 1.0)
        # first weight tiles of fc3 (DMA'd mid-fc1 so they don't delay the
        # startup x/w1 streams)
        w3pre = {}
        for i in (4, 7):
            w3pre[i] = cpool.tile([128, KBS, 2, 128], dt.float8e4,
                                  tag=f"w3pre{i}", name=f"w3pre{i}")
        warm = cpool.tile([1, 1], dt.float32)

        # ---- PE pre-warm: no-DMA matmuls so the HAM clock-gate is at
        # 8/8 by the time the first real matmul issues ----
        wmw = cpool.tile([128, 128], dt.float16)
        nc.vector.memset(wmw[:], 0.0)
        wmm = cpool.tile([128, 128], dt.float16)
        nc.vector.memset(wmm[:], 0.0)
        wp = warmps.tile([128, 128], dt.float32)
        for _ in range(NWARM):
            nc.tensor.matmul(wp[:], wmw[:], wmm[:], start=True, stop=True)
        nc.scalar.activation(warm[:], wp[0:1, 0:1], AF.Identity)

        # ============================ fc1 ============================
        h1rp = tc.alloc_tile_pool(name="h1rp", bufs=1)
        h1R = h1rp.tile([128, NF, 512], dt.float8e4)
        stg1p = tc.alloc_tile_pool(name="stg1p", bufs=1)
        xtq = tc.alloc_tile_pool(name="xtq", bufs=1)
        w1pool = tc.alloc_tile_pool(name="w1pool", bufs=2)
        ps1 = tc.alloc_tile_pool(name="ps1", bufs=2, space="PSUM")

        # startup DMAs in fc1 consumption order (k-interleaved),
        # alternating dispatch engines
        w1q0 = {}
        xt_half = {}
        xt_full = {}
        qs = [nc.sync, nc.gpsimd, nc.scalar]
        for k in range(KT1):
            # halves across the three DMA-capable queues so the first
            # m-iterations never wait on the x stream
            tx = xtq.tile([128, BC], dt.float16, tag=f"xt_{k}")
            for n in range(NH):
                qs[(2 * k + n) % 3].dma_start(
                    out=tx[:, n * 512:(n + 1) * 512],
                    in_=xt[:, k, n * 512:(n + 1) * 512])
                xt_half[(k, n)] = tx[:, n * 512:(n + 1) * 512]
            xt_full[k] = tx[:, :]
            tw = xtq.tile([128, MPQ * 128], dt.float16, tag=f"w1q0_{k}")
            (nc.scalar if k < 4 else nc.gpsimd).dma_start(
                out=tw[:], in_=w1t[0, :, k, :])
            w1q0[k] = tw

        def s2_chunks(kk, sl):
            """Build chunk kk of all 10 fc2 S-tiles. sl = row 24+kk left
            half (A12); h1L/h1R rows 0..23; a22r1 row kk = A22."""
            for px in range(2):
                c0 = px * 256
                L = h1L[:, kk, c0:c0 + 256]
                R = h1R[:, kk, c0:c0 + 256]
                A12 = sl[:, c0:c0 + 256]
                A22 = a22r1[:, kk, c0:c0 + 256]
                tt(S2[(7, px)][:, kk, :], A12, A22, SUB)
                tt(S2[(5, px)][:, kk, :], L, A12, ADD)
                tt(S2[(1, px)][:, kk, :], L, A22, ADD)
                tt(S2[(2, px)][:, kk, :], R, A22, ADD)
                tt(S2[(6, px)][:, kk, :], R, L, SUB)

        w2f0 = {}
        w3f0_pre = {}

        def w2_prefetch(idxs, eng):
            for i in idxs:
                t = w2pool.tile([128, KBS, 2, 128], dt.float8e4, tag="w2")
                eng.dma_start(out=t[:], in_=w2s[0, :, i - 1])
                w2f0[i] = t

        for q in range(MQ):
            if q == 0:
                def lhs1(k, mi):
                    return w1q0[k][:, mi * 128:(mi + 1) * 128]
            else:
                w1q = w1pool.tile([128, KT1, MPQ * 128], dt.float16, tag="w1")
                nc.sync.dma_start(out=w1q[:], in_=w1t[q])

                def lhs1(k, mi, w1q=w1q):
                    return w1q[:, k, mi * 128:(mi + 1) * 128]
            if q == 6:
                for i, eng in ((4, nc.sync), (7, nc.gpsimd)):
                    eng.dma_start(out=w3pre[i][:], in_=w3s[0, :, i - 1])
            elif q == 12:
                w2_prefetch((4, 7, 5), nc.gpsimd)
            elif q == 18:
                w2_prefetch((1, 2, 3), nc.sync)
            for mi in range(MPQ):
                m = q * MPQ + mi
                psum = ps1.tile([128, BC], dt.float32, tag="ps1")
                for k in range(KT1):
                    for n in range(NH):
                        nc.tensor.matmul(
                            psum[:, n * 512:(n + 1) * 512],
                            lhs1(k, mi),
                            xt_half[(k, n)],
                            start=(k == 0),
                            stop=(k == KT1 - 1),
                        )
                if m < NF:
                    nc.scalar.sign(h1L[:, m, :], psum[:, 0:512],
                                   bias=b1_sb[:, m:m + 1])
                    nc.scalar.sign(h1R[:, m, :], psum[:, 512:BC],
                                   bias=b1_sb[:, m:m + 1])
                else:
                    kk = m - NF
                    sl = stg1p.tile([128, 512], dt.float8e4,
                                    tag=f"s1stg{m % 2}", name="s1stg")
                    nc.scalar.sign(sl[:], psum[:, 0:512],
                                   bias=b1_sb[:, m:m + 1])
                    nc.scalar.sign(a22r1[:, kk, :], psum[:, 512:BC],
                                   bias=b1_sb[:, m:m + 1])
                    s2_chunks(kk, sl)

        ps1.release()
        w1pool.release()
        xtq.release()
        stg1p.release()
        h1rp.release()

        # ---------------- right-stack pools: fc2 outputs ----------------
        # Live from here to the end of fc3; the left stack keeps cycling
        # its phase pools underneath.
        s3pool = tc.alloc_tile_pool(name="s3pool", bufs=1, side="right")
        h2lp = tc.alloc_tile_pool(name="h2lp", bufs=1, side="right")
        a22r2p = tc.alloc_tile_pool(name="a22r2p", bufs=1, side="right")
        stg2p = tc.alloc_tile_pool(name="stg2p", bufs=1, side="right")

        S3 = {}
        for px in range(2):
            for i in (1, 2, 5, 6, 7):
                S3[(i, px)] = s3pool.tile([128, NF, 256], dt.float8e4,
                                          tag=f"s3_{i}_{px}",
                                          name=f"s3_{i}_{px}")
        h2L = h2lp.tile([128, NF, 512], dt.float8e4)
        a22r2 = a22r2p.tile([128, NF, 512], dt.float8e4)

        # ------------- fc2 (one-level Strassen) -------------
        # lgp (fc4's logit accumulator) sits below psp on the PSUM stack so
        # psp can release before the tail needs its banks for transposes.
        # fc2+fc3 share the one product-psum pool (same tags) so fc3's
        # first matmul has no cross-pool PSUM overlap dependency.
        lgp = tc.alloc_tile_pool(name="lgp", bufs=1, space="PSUM")
        lg_psum = lgp.tile([DOUT, BC], dt.float32)
        ps2 = tc.alloc_tile_pool(name="psp", bufs=1, space="PSUM")

        dmae = [nc.sync, nc.gpsimd]
        nd = [0]

        def s3_parts(f, px, stg21, stg12):
            """Per-op builders for the px-quarter of chunk f of the fc3
            S-tiles, emitted as soon as their sign inputs exist: h2L row f
            = A11 (i=1), stg21 = A21 (i=2), stg12 = A12 (i=3), a22r2 row f
            = A22 (i=6)."""
            c0 = px * 256
            L = h2L[:, f, c0:c0 + 256]
            A21 = stg21[:, c0:c0 + 256]
            A12 = stg12[:, c0:c0 + 256]
            A22 = a22r2[:, f, c0:c0 + 256]
            return {
                2: lambda: tt(S3[(6, px)][:, f, :], A21, L, SUB),
                3: lambda: tt(S3[(5, px)][:, f, :], L, A12, ADD),
                6: lambda: (tt(S3[(7, px)][:, f, :], A12, A22, SUB),
                            tt(S3[(1, px)][:, f, :], L, A22, ADD),
                            tt(S3[(2, px)][:, f, :], A21, A22, ADD)),
            }

        for f in range(NF):
            wts = {}
            if f == 0:
                wts.update(w2f0)
                for i in (6,):
                    t = w2pool.tile([128, KBS, 2, 128], dt.float8e4, tag="w2")
                    eng = dmae[nd[0] % 2]
                    nd[0] += 1
                    eng.dma_start(out=t[:], in_=w2s[0, :, i - 1])
                    wts[i] = t
            else:
                for i in ORDER:
                    t = w2pool.tile([128, KBS, 2, 128], dt.float8e4, tag="w2")
                    eng = dmae[nd[0] % 2]
                    nd[0] += 1
                    eng.dma_start(out=t[:], in_=w2s[f, :, i - 1])
                    wts[i] = t
                if f == NF - 1:
                    # fc3's f=0 weight tiles ride the (still-live) w2pool:
                    # their DMAs drain during this iteration's compute, so
                    # the fc2->fc3 switch never waits on weight DMA
                    for i in (5, 1, 2, 3, 6):
                        t = w2pool.tile([128, KBS, 2, 128], dt.float8e4,
                                        tag="w2")
                        eng = dmae[nd[0] % 2]
                        nd[0] += 1
                        eng.dma_start(out=t[:], in_=w3s[0, :, i - 1])
                        w3f0_pre[i] = t
            stg21 = stg2p.tile([128, 512], dt.float8e4,
                               tag=f"s21_{f % 2}", name="s21")
            stg12 = stg2p.tile([128, 512], dt.float8e4,
                               tag=f"s12_{f % 2}", name="s12")
            for px in range(2):
                c0 = px * 256
                s3b = s3_parts(f, px, stg21, stg12)

                def moving2(i, blk, c0=c0, px=px):
                    if i == 3:
                        return h1L[:, 2 * blk:2 * blk + 2, c0:c0 + 256]
                    if i == 4:
                        return a22r1[:, 2 * blk:2 * blk + 2, c0:c0 + 256]
                    return S2[(i, px)][:, 2 * blk:2 * blk + 2, :]
                # 7 quarter-size products packed pairwise into 2KB PSUM
                # banks; pairs chosen by lifetime so bufs=1 tags never
                # stall the PE
                pA = ps2.tile([128, 512], dt.float32, tag="pA", name="pA")
                pB = ps2.tile([128, 512], dt.float32, tag="pB", name="pB")
                pC = ps2.tile([128, 512], dt.float32, tag="pC", name="pC")
                pD = ps2.tile([128, 256], dt.float32, tag="pD", name="pD",
                              bufs=2)
                psm = {4: pA[:, 0:256], 7: pA[:, 256:512],
                       5: pB[:, 0:256], 1: pB[:, 256:512],
                       2: pC[:, 0:256], 3: pC[:, 256:512],
                       6: pD[:, 0:256]}

                def ctile(tag, bufs=1):
                    return c2pool.tile([128, 256], dt.float32,
                                       tag=tag, name=tag, bufs=bufs)
                for i in ORDER:
                    for blk in range(KBS):
                        nc.tensor.matmul(
                            psm[i], wts[i][:, blk],
                            moving2(i, blk),
                            start=(blk == 0),
                            stop=(blk == KBS - 1),
                            perf_mode=DR,
                        )
                    if i == 4:
                        m4s = ctile("m4s")
                        nc.scalar.activation(m4s[:], psm[4], AF.Identity)
                    elif i == 7:
                        x1 = ctile("t_a")
                        tt(x1[:], m4s[:], psm[7], ADD)
                    elif i == 5:
                        m5s = ctile("m5s")
                        nc.scalar.activation(m5s[:], psm[5], AF.Identity)
                    elif i == 1:
                        x2 = ctile("t_b")
                        tt(x2[:], x1[:], psm[1], ADD)
                        c11 = ctile("c11")
                        tt(c11[:], x2[:], m5s[:], SUB)
                        nc.scalar.sign(h2L[:, f, c0:c0 + 256], c11[:],
                                       bias=b2_sb[:, f:f + 1])
                    elif i == 2:
                        c21 = ctile("c21")
                        tt(c21[:], m4s[:], psm[2], ADD)
                        nc.scalar.sign(stg21[:, c0:c0 + 256], c21[:],
                                       bias=b2_sb[:, f:f + 1])
                        m2s = ctile("m2s")
                        nc.scalar.activation(m2s[:], psm[2], AF.Identity)
                        s3b[2]()
                    elif i == 3:
                        c12 = ctile("c12")
                        tt(c12[:], m5s[:], psm[3], ADD)
                        nc.scalar.sign(stg12[:, c0:c0 + 256], c12[:],
                                       bias=b2_sb[:, 24 + f:25 + f])
                        s3b[3]()
                        y2 = ctile("t_a")
                        nc.vector.scalar_tensor_tensor(
                            y2[:], m2s[:], -1.0, psm[1], MULT, ADD)
                        y3 = ctile("t_b")
                        tt(y3[:], y2[:], psm[3], ADD)
                    elif i == 6:
                        c22 = ctile("c22")
                        tt(c22[:], y3[:], psm[6], ADD)
                        nc.scalar.sign(a22r2[:, f, c0:c0 + 256], c22[:],
                                       bias=b2_sb[:, 24 + f:25 + f])
                        s3b[6]()

        s2pool.release()
        a22r1p.release()
        h1lp.release()

        # ------------- fc3 (Strassen) + fused fc4 -------------
        if True:
            with tc.tile_pool(name="w3pool", bufs=8) as w3pool, \
                 tc.tile_pool(name="c3pool", bufs=1) as cp3, \
                 tc.tile_pool(name="h3pool", bufs=8) as h3pool:
                ps3 = ps2
                dmae3 = [nc.sync, nc.gpsimd]
                nd3 = [0]
                w3f0 = dict(w3pre)
                w3f0.update(w3f0_pre)

                h3_tiles = {}

                def fc4_mms(m):
                    t_h3 = h3_tiles[m]
                    for n in range(NH):
                        nc.tensor.matmul(
                            lg_psum[:, n * 512:(n + 1) * 512],
                            w4_sb[:, m, :],
                            t_h3[:, n * 512:(n + 1) * 512],
                            start=(m == 0),
                            stop=(m == MT - 1),
                        )

                def h3_store(m, cols, csb):
                    sl = h3_tiles[m][:, cols[0]:cols[1]]
                    nc.scalar.activation(sl, csb[:], AF.Identity,
                                         bias=b3_sb[:, m:m + 1])
                    nc.vector.tensor_scalar(sl, sl, 1.0, -1.0,
                                            mybir.AluOpType.min,
                                            mybir.AluOpType.max)

                # fc3 loops: f outer (so h3/fc4 drain promptly), px inner
                for f in range(NF):
                    for m in (f, 24 + f):
                        h3_tiles[m] = h3pool.tile([128, BC], dt.float16,
                                                  tag="h3", name="h3")
                    wts = {}
                    if f == 0:
                        wts = w3f0
                    else:
                        for i in ORDER:
                            t = w3pool.tile([128, KBS, 2, 128],
                                            dt.float8e4, tag="w3")
                            eng = dmae3[nd3[0] % 2]
                            nd3[0] += 1
                            eng.dma_start(out=t[:], in_=w3s[f, :, i - 1])
                            wts[i] = t
                    for px in range(2):
                        c0 = px * 256

                        def moving3(i, blk, c0=c0, px=px):
                            if i == 3:
                                return h2L[:, 2 * blk:2 * blk + 2,
                                           c0:c0 + 256]
                            if i == 4:
                                return a22r2[:, 2 * blk:2 * blk + 2,
                                             c0:c0 + 256]
                            return S3[(i, px)][:, 2 * blk:2 * blk + 2, :]
                        qA = ps3.tile([128, 512], dt.float32,
                                      tag="pA", name="qA")
                        qB = ps3.tile([128, 512], dt.float32,
                                      tag="pB", name="qB")
                        qC = ps3.tile([128, 512], dt.float32,
                                      tag="pC", name="qC")
                        qD = ps3.tile([128, 256], dt.float32,
                                      tag="pD", name="qD", bufs=2)
                        psm = {4: qA[:, 0:256], 7: qA[:, 256:512],
                               5: qB[:, 0:256], 1: qB[:, 256:512],
                               2: qC[:, 0:256], 3: qC[:, 256:512],
                               6: qD[:, 0:256]}

                        def ctile3(tag, bufs=1):
                            return cp3.tile([128, 256], dt.float32,
                                            tag=tag, name=tag, bufs=bufs)
                        for i in ORDER:
                            for blk in range(KBS):
                                nc.tensor.matmul(
                                    psm[i], wts[i][:, blk],
                                    moving3(i, blk),
                                    start=(blk == 0),
                                    stop=(blk == KBS - 1),
                                    perf_mode=DR,
                                )
                            if i == 4:
                                m4s = ctile3("f3m4s")
                                nc.scalar.activation(m4s[:], psm[4],
                                                     AF.Identity)
                            elif i == 7:
                                x1 = ctile3("f3x1")
                                tt(x1[:], m4s[:], psm[7], ADD)
                            elif i == 5:
                                m5s = ctile3("f3m5s")
                                nc.scalar.activation(m5s[:], psm[5],
                                                     AF.Identity)
                            elif i == 1:
                                x2 = ctile3("f3x2")
                                tt(x2[:], x1[:], psm[1], ADD)
                                c11 = ctile3("f3c11", 2)
                                tt(c11[:], x2[:], m5s[:], SUB)
                                h3_store(f, (c0, c0 + 256), c11)
                            elif i == 2:
                                c21 = ctile3("f3c21", 2)
                                tt(c21[:], m4s[:], psm[2], ADD)
                                h3_store(f, (512 + c0, 512 + c0 + 256), c21)
                                m2s = ctile3("f3m2s")
                                nc.scalar.activation(m2s[:], psm[2],
                                                     AF.Identity)
                                if f == NF - 1 and px == 1:
                                    fc4_mms(f)
                            elif i == 3:
                                c12 = ctile3("f3c12", 2)
                                tt(c12[:], m5s[:], psm[3], ADD)
                                h3_store(24 + f, (c0, c0 + 256), c12)
                                if f == NF - 1 and px == 1:
                                    nc.tensor.matmul(
                                        lg_psum[:, 0:512],
                                        w4_sb[:, 24 + f, :],
                                        h3_tiles[24 + f][:, 0:512],
                                        start=False, stop=True)
                                y2 = ctile3("f3y2")
                                nc.vector.scalar_tensor_tensor(
                                    y2[:], m2s[:], -1.0, psm[1],
                                    MULT, ADD)
                                y3 = ctile3("f3y3")
                                tt(y3[:], y2[:], psm[3], ADD)
                            elif i == 6:
                                c22 = ctile3("f3c22", 2)
                                tt(c22[:], y3[:], psm[6], ADD)
                                h3_store(24 + f,
                                         (512 + c0, 512 + c0 + 256), c22)
                                if f == NF - 1 and px == 1:
                                    nc.tensor.matmul(
                                        lg_psum[:, 512:BC],
                                        w4_sb[:, 24 + f, :],
                                        h3_tiles[24 + f][:, 512:BC],
                                        start=False, stop=True)
                    # fc4, pipelined one f behind
                    if f > 0:
                        fc4_mms(f - 1)
                        fc4_mms(24 + f - 1)
                    if f == NF - 2:
                        # pull the Exp table into the ACT engine well ahead
                        # of the tail (a function switch costs a 1.28us
                        # ACT_TABLE_LOAD; the last f's ACT queue is too
                        # backlogged to absorb it)
                        nc.scalar.activation(warm[:], ident[0:1, 0:1],
                                             AF.Exp)
            ps2.release()

            # ------------- bias + log_softmax (max-free) -------------
            # logits are bounded (|l| < 40), so exp() cannot overflow fp32
            # and the rowmax subtraction is unnecessary:
            # out = l - ln(sum(exp(l))). Partition 32 of lg_sb holds the
            # per-column ln-sum so one PE transpose carries both. The Exp
            # activation table was pre-warmed during the last fc4 matmuls,
            # so only the Ln switch pays a table load here.
            NJ = BC // 128
            with tc.tile_pool(name="tp", bufs=1, space="PSUM") as tpp, \
                 tc.tile_pool(name="sm", bufs=1) as smp:
                lg_sb = smp.tile([33, BC], dt.float32)
                ex_sb = smp.tile([DOUT, BC], dt.float16, tag="ex")
                sums_ps = tpp.tile([1, BC], dt.float32, tag="sums")
                # halved exp -> sums -> ln chain so each stage starts as
                # soon as half its input exists
                for n in range(NH):
                    nc.scalar.activation(ex_sb[:, n * 512:(n + 1) * 512],
                                         lg_psum[:, n * 512:(n + 1) * 512],
                                         AF.Exp, bias=b4_sb[:, 0:1])
                    if n == 0:
                        # pull the Ln table in behind the Exps so the real
                        # Ln doesn't pay the 1.28us load serially
                        nc.scalar.activation(warm[:], warm[:], AF.Ln)
                # biased logits copy runs on the DVE, in parallel with the
                # ACT engine's Exp
                nc.vector.tensor_scalar(lg_sb[0:DOUT, :], lg_psum[:],
                                        b4_sb[:, 0:1], None, ADD)
                for n in range(NH):
                    nc.tensor.matmul(
                        sums_ps[:, n * 512:(n + 1) * 512],
                        ones_sb[:, 0:1],
                        ex_sb[:, n * 512:(n + 1) * 512],
                    )
                    nc.scalar.activation(
                        lg_sb[32:33, n * 512:(n + 1) * 512],
                        sums_ps[:, n * 512:(n + 1) * 512], AF.Ln)
                dmo = [nc.sync, nc.sync]
                for j in range(NJ):
                    tp = tpp.tile([128, 33], dt.float32, tag=f"tp{j % 2}")
                    nc.tensor.transpose(
                        tp[:], lg_sb[:, j * 128:(j + 1) * 128], ident[:])
                    res = smp.tile([128, DOUT], dt.float32, tag=f"res{j}")
                    nc.vector.tensor_scalar(res[:], tp[:, 0:DOUT],
                                            tp[:, 32:33], None, SUB)
                    dmo[j % 2].dma_start(
                        out=out[j * 128:(j + 1) * 128, :], in_=res[:])

        # releases (reverse open order per side)
        lgp.release()
        c2pool.release()
        w2pool.release()
        stg2p.release()
        a22r2p.release()
        h2lp.release()
        s3pool.release()
        warmps.release()
        cpool.release()

    nc.compile()
    return nc


def _pack_inputs(x, w1, b1, w2, b2, w3, b3, w4, b4):
    """Host-side packing into the device layouts. Shared tensors are packed
    once; only xt differs per core."""
    f32 = np.float32
    f16 = np.float16
    x = np.asarray(x, f32).reshape(B, DIN)

    # fc1 weights: sign(w1).T stacked twice (hi/lo terms share the weights),
    # padded to [1664, 6144], layout [q, p, k, m]
    s1 = np.sign(np.asarray(w1, f32))                       # [DH, DIN]
    s1t = np.zeros((K1P, DH), f16)
    s1t[:DIN] = s1.T
    s1t[DIN:2 * DIN] = s1.T
    w1t = np.ascontiguousarray(
        s1t.reshape(KT1, 128, MQ, MPQ * 128).transpose(2, 1, 0, 3))

    # fc2/fc3 weights: Strassen T-combos of sign(w).T, DoubleRow layout per
    # 128-wide output chunk: [fo, p, 7, blk, i2, f']
    def pack_strassen(w):
        sm = np.sign(np.asarray(w, f32)).T                  # [in, out] = B
        H = DH // 2
        B11 = sm[:H, :H]
        B12 = sm[:H, H:]
        B21 = sm[H:, :H]
        B22 = sm[H:, H:]
        Ts = [B11 + B22, B11, B12 - B22, B21 - B11, B22, B11 + B12,
              B21 + B22]

        def pack_t(t):   # [3072, 3072] -> [fo, p, blk, i2, f']
            r = t.reshape(KBS, 2, 128, NF, 128)
            return r.transpose(3, 2, 0, 1, 4)

        return np.ascontiguousarray(
            np.stack([pack_t(t) for t in Ts], axis=2)).astype(FP8)

    w2sp = pack_strassen(w2)
    w3sp = pack_strassen(w3)

    # fc4 weights: w4.T in fp16, layout [p, j, c]
    w4t = np.asarray(w4, f32).T.astype(f16)                 # [DH, DOUT]
    w4pk = np.ascontiguousarray(w4t.reshape(MT, 128, DOUT).transpose(1, 0, 2))

    def pack_b(b):
        return np.ascontiguousarray(np.asarray(b, f32).reshape(MT, 128).T)

    b1pk, b2pk, b3pk = pack_b(b1), pack_b(b2), pack_b(b3)
    b4pk = np.asarray(b4, f32).reshape(DOUT, 1)

    shared = {"w1t": w1t, "w2s": w2sp, "w3s": w3sp, "w4p": w4pk,
              "b1p": b1pk, "b2p": b2pk, "b3p": b3pk, "b4p": b4pk}

    # per-core x: fp16 hi/lo split stacked along contraction, layout [p, k, n]
    in_maps = []
    for c in range(CORES):
        xc = x[c * BC:(c + 1) * BC]                         # [BC, DIN]
        hi = xc.astype(f16)
        lo = (xc - hi.astype(f32)).astype(f16)
        arr = np.zeros((K1P, BC), f16)
        arr[:DIN] = hi.T
        arr[DIN:2 * DIN] = lo.T
        xtc = np.ascontiguousarray(arr.reshape(KT1, 128, BC).transpose(1, 0, 2))
        in_maps.append({"xt": xtc, **shared})
    return in_maps


_cached_nc = None


def kernel(x, w1, b1, w2, b2, w3, b3, w4, b4):
    global _cached_nc, last_exec_time_ns
    import os
    trace = bool(int(os.environ.get("KERNEL_TRACE", "0")))
    if _cached_nc is None:
        _cached_nc = _build_program()
    in_maps = _pack_inputs(x, w1, b1, w2, b2, w3, b3, w4, b4)
    res = run_bass_kernel_spmd(_cached_nc, in_maps, list(range(CORES)),
                               trace=trace)
    last_exec_time_ns = res.exec_time_ns
    return np.concatenate([res.results[c]["out"] for c in range(CORES)], axis=0)


# revision 5
# speedup vs baseline: 1.0362x; 1.0362x over previous
"""Binarized 4-layer MLP (8192x784 -> 6144 -> 6144 -> 6144 -> 10, log_softmax)
on 8 Trainium2 NeuronCores, data-parallel over the batch.

Per-core dataflow (batch slice of 1024, feature-major activations [feat, batch]):
  fc1: x @ sign(w1).T as a 2-term fp16 hi/lo split of x, with the two terms
       stacked along the contraction dim (1568 rows -> 13 k-tiles). fp16
       upconverts losslessly to the PE's e10m11 internal format and the
       weights are exactly +-1, so this reproduces fp32 accuracy.
  fc2/fc3: one level of Strassen over the 1024x6144 @ 6144x6144 binary
       matmul: 7 half-size products instead of 8 (12.5% fewer PE cycles).
       Activation-side combos are {-2,0,2} (exact in fp8, built on the DVE);
       weight-side combos are precomputed on the host. Products run in fp8e4
       DoubleRow; all values are small integers so fp32 PSUM stays bit-exact.
  fc4: fused into the fc3 f-loop, single fp16 pass.
  log_softmax: max-free, transpose-first: per 128-batch chunk, PE-transpose
       the 10 logits, then Exp/reduce/Ln/subtract run on [128,10] tiles.

Scheduling (v2): every Strassen S-tile (10 for fc2, 10 for fc3) is
chunk-built on the DVE inside the loop that produces its inputs, so the
fc1->fc2 and fc2->fc3 phase boundaries have no serial whole-tile builds.
To make the 20 S-tiles fit, raw h1/h2 are never materialized: only the
quarters the direct Strassen products read (A11-left, A22-right) persist,
plus a 2-row staging buffer for combo inputs. fc2-output-side tiles live on
the right SBUF stack so their lifetime (mid-fc2 -> fc3-end) can overlap the
left stack's phase pools. First-f weight tiles for fc2/fc3 are prefetched /
JIT-loaded so the PE never waits on weight DMA at a phase switch. ~150
dummy matmuls at t=0 hold the PE busy so the HAM clock-gate is warm when
the first real matmul issues.
"""

import numpy as np
import ml_dtypes

import concourse.bass as bass
import concourse.mybir as mybir
from concourse import bacc
from concourse.tile import TileContext
from concourse.bass_utils import run_bass_kernel_spmd
from concourse.masks import make_identity

dt = mybir.dt

CORES = 8
B = 8192
BC = B // CORES          # 1024 batch rows per core
DIN = 784
KT1 = 13                 # fc1 contraction tiles: 2*784 = 1568 padded to 1664
K1P = KT1 * 128
DH = 6144
MT = DH // 128           # 48 feature tiles
KBS = 12                 # DoubleRow blocks per 3072 Strassen half
NF = 24                  # 128-wide output chunks per 3072 block-column
DOUT = 10
NH = BC // 512           # 2 moving halves of 512
MQ = 24                  # fc1 m-groups (w1 streamed per 2 m-tiles)
MPQ = MT // MQ
NWARM = 150              # PE pre-warm matmuls (FD=128) to beat the HAM gate

BF16 = ml_dtypes.bfloat16
FP8 = mybir.dt.np(dt.float8e4)

last_exec_time_ns = None

# Strassen product order: direct-operand chains (M4) first; combines fire as
# soon as their inputs exist. M3/M4 read the persisted quarters directly,
# the rest read S-combos.
ORDER = [4, 7, 5, 1, 2, 3, 6]


def _build_program():
    nc = bacc.Bacc("TRN2", target_bir_lowering=False, debug=False,
                   num_devices=CORES)

    xt = nc.dram_tensor("xt", [128, KT1, BC], dt.float16,
                        kind="ExternalInput").ap()
    w1t = nc.dram_tensor("w1t", [MQ, 128, KT1, MPQ * 128], dt.float16,
                         kind="ExternalInput").ap()
    w2s = nc.dram_tensor("w2s", [NF, 128, 7, KBS, 2, 128], dt.float8e4,
                         kind="ExternalInput").ap()
    w3s = nc.dram_tensor("w3s", [NF, 128, 7, KBS, 2, 128], dt.float8e4,
                         kind="ExternalInput").ap()
    w4p = nc.dram_tensor("w4p", [128, NF, 2, 16], dt.float8e4,
                         kind="ExternalInput").ap()
    b1p = nc.dram_tensor("b1p", [128, MT], dt.float32, kind="ExternalInput").ap()
    b2p = nc.dram_tensor("b2p", [128, MT], dt.float32, kind="ExternalInput").ap()
    b3p = nc.dram_tensor("b3p", [128, MT], dt.float32, kind="ExternalInput").ap()
    b4p = nc.dram_tensor("b4p", [DOUT, 1], dt.float32, kind="ExternalInput").ap()
    out = nc.dram_tensor("out", [BC, DOUT], dt.float32, kind="ExternalOutput").ap()

    DR = mybir.MatmulPerfMode.DoubleRow
    AF = mybir.ActivationFunctionType
    ADD = mybir.AluOpType.add
    SUB = mybir.AluOpType.subtract
    MULT = mybir.AluOpType.mult

    def tt(out_, a, b_, op):
        nc.vector.scalar_tensor_tensor(out_, a, 1.0, b_, MULT, op)

    with TileContext(nc) as tc:
        # ---------------- persistent pools (left stack) ----------------
        # w2pool/c2pool sit at the BOTTOM of the stack (below the fc1-side
        # tiles) and stay alive through fc3, so the fc3 pools allocate in
        # the region freed by s2pool/h1-quarters, whose release only waits
        # on PE readers (not the slow DVE combine drain).
        cpool = tc.alloc_tile_pool(name="consts", bufs=1)
        w2pool = tc.alloc_tile_pool(name="w2pool", bufs=7)
        c2pool = tc.alloc_tile_pool(name="c2pool", bufs=1)
        # fc1-output side, live fc1+fc2: A11|A21 rows (0..23) split L/R,
        # A22 right quarter (rows 24..47), all fp8 sign outputs.
        h1lp = tc.alloc_tile_pool(name="h1lp", bufs=1)
        a22r1p = tc.alloc_tile_pool(name="a22r1p", bufs=1)
        s2pool = tc.alloc_tile_pool(name="s2pool", bufs=1)
        # PE warm-up PSUM target on the right PSUM stack (never conflicts
        # with the phase psum pools).
        warmps = tc.alloc_tile_pool(name="warmps", bufs=1, space="PSUM",
                                    side="right")

        h1L = h1lp.tile([128, NF, 512], dt.float8e4)
        a22r1 = a22r1p.tile([128, NF, 512], dt.float8e4)
        S2 = {}
        for px in range(2):
            for i in (1, 2, 5, 6, 7):
                S2[(i, px)] = s2pool.tile([128, NF, 256], dt.float8e4,
                                          tag=f"s2_{i}_{px}",
                                          name=f"s2_{i}_{px}")

        # consts
        b1_sb = cpool.tile([128, MT], dt.float32)
        nc.sync.dma_start(out=b1_sb[:], in_=b1p[:])
        b2_sb = cpool.tile([128, MT], dt.float32)
        nc.gpsimd.dma_start(out=b2_sb[:], in_=b2p[:])
        b3_sb = cpool.tile([128, MT], dt.float32)
        nc.sync.dma_start(out=b3_sb[:], in_=b3p[:])
        b4_sb = cpool.tile([DOUT, 1], dt.float32)
        nc.gpsimd.dma_start(out=b4_sb[:], in_=b4p[:])
        w4_sb = cpool.tile([128, NF, 2, 16], dt.float8e4)
        nc.sync.dma_start(out=w4_sb[:], in_=w4p[:])
        ident = cpool.tile([33, 33], dt.float32)
        make_identity(nc, ident[:])
        ones_sb = cpool.tile([DOUT, 1], dt.float16)
        nc.vector.memset(ones_sb[:], 1.0)
        # first weight tiles of fc3 (DMA'd mid-fc1 so they don't delay the
        # startup x/w1 streams)
        w3pre = {}
        for i in (4, 7):
            w3pre[i] = cpool.tile([128, KBS, 2, 128], dt.float8e4,
                                  tag=f"w3pre{i}", name=f"w3pre{i}")
        warm = cpool.tile([1, 1], dt.float32)

        # ---- PE pre-warm: no-DMA matmuls so the HAM clock-gate is at
        # 8/8 by the time the first real matmul issues ----
        wmw = cpool.tile([128, 128], dt.float16)
        nc.vector.memset(wmw[:], 0.0)
        wmm = cpool.tile([128, 128], dt.float16)
        nc.vector.memset(wmm[:], 0.0)
        wp = warmps.tile([128, 64], dt.float32)
        for _ in range(NWARM):
            nc.tensor.matmul(wp[:], wmw[:], wmm[:, 0:64], start=True,
                             stop=True)
        nc.scalar.activation(warm[:], wp[0:1, 0:1], AF.Identity)

        # ============================ fc1 ============================
        h1rp = tc.alloc_tile_pool(name="h1rp", bufs=1)
        h1R = h1rp.tile([128, NF, 512], dt.float8e4)
        stg1p = tc.alloc_tile_pool(name="stg1p", bufs=1)
        xtq = tc.alloc_tile_pool(name="xtq", bufs=1)
        w1pool = tc.alloc_tile_pool(name="w1pool", bufs=2)
        ps1 = tc.alloc_tile_pool(name="ps1", bufs=2, space="PSUM")

        # startup DMAs in fc1 consumption order (k-interleaved),
        # alternating dispatch engines
        w1q0 = {}
        xt_half = {}
        xt_full = {}
        qs = [nc.sync, nc.gpsimd, nc.scalar]
        for k in range(KT1):
            # halves across the three DMA-capable queues so the first
            # m-iterations never wait on the x stream
            tx = xtq.tile([128, BC], dt.float16, tag=f"xt_{k}")
            for n in range(NH):
                qs[(2 * k + n) % 3].dma_start(
                    out=tx[:, n * 512:(n + 1) * 512],
                    in_=xt[:, k, n * 512:(n + 1) * 512])
                xt_half[(k, n)] = tx[:, n * 512:(n + 1) * 512]
            xt_full[k] = tx[:, :]
            tw = xtq.tile([128, MPQ * 128], dt.float16, tag=f"w1q0_{k}")
            (nc.scalar if k < 4 else nc.gpsimd).dma_start(
                out=tw[:], in_=w1t[0, :, k, :])
            w1q0[k] = tw

        def s2_chunks(kk, sl):
            """Build chunk kk of all 10 fc2 S-tiles. sl = row 24+kk left
            half (A12); h1L/h1R rows 0..23; a22r1 row kk = A22."""
            for px in range(2):
                c0 = px * 256
                L = h1L[:, kk, c0:c0 + 256]
                R = h1R[:, kk, c0:c0 + 256]
                A12 = sl[:, c0:c0 + 256]
                A22 = a22r1[:, kk, c0:c0 + 256]
                tt(S2[(7, px)][:, kk, :], A12, A22, SUB)
                tt(S2[(5, px)][:, kk, :], L, A12, ADD)
                tt(S2[(1, px)][:, kk, :], L, A22, ADD)
                tt(S2[(2, px)][:, kk, :], R, A22, ADD)
                tt(S2[(6, px)][:, kk, :], R, L, SUB)

        w2f0 = {}
        w3f0_pre = {}

        def w2_prefetch(idxs, eng):
            for i in idxs:
                t = w2pool.tile([128, KBS, 2, 128], dt.float8e4, tag="w2")
                eng.dma_start(out=t[:], in_=w2s[0, :, i - 1])
                w2f0[i] = t

        for q in range(MQ):
            if q == 0:
                def lhs1(k, mi):
                    return w1q0[k][:, mi * 128:(mi + 1) * 128]
            else:
                w1q = w1pool.tile([128, KT1, MPQ * 128], dt.float16, tag="w1")
                nc.sync.dma_start(out=w1q[:], in_=w1t[q])

                def lhs1(k, mi, w1q=w1q):
                    return w1q[:, k, mi * 128:(mi + 1) * 128]
            if q == 6:
                for i, eng in ((4, nc.sync), (7, nc.gpsimd)):
                    eng.dma_start(out=w3pre[i][:], in_=w3s[0, :, i - 1])
            elif q == 12:
                w2_prefetch((4, 7, 5), nc.gpsimd)
            elif q == 18:
                w2_prefetch((1, 2, 3), nc.sync)
            for mi in range(MPQ):
                m = q * MPQ + mi
                psum = ps1.tile([128, BC], dt.float32, tag="ps1")
                for k in range(KT1):
                    for n in range(NH):
                        nc.tensor.matmul(
                            psum[:, n * 512:(n + 1) * 512],
                            lhs1(k, mi),
                            xt_half[(k, n)],
                            start=(k == 0),
                            stop=(k == KT1 - 1),
                        )
                if m < NF:
                    nc.scalar.sign(h1L[:, m, :], psum[:, 0:512],
                                   bias=b1_sb[:, m:m + 1])
                    nc.scalar.sign(h1R[:, m, :], psum[:, 512:BC],
                                   bias=b1_sb[:, m:m + 1])
                else:
                    kk = m - NF
                    sl = stg1p.tile([128, 512], dt.float8e4,
                                    tag=f"s1stg{m % 2}", name="s1stg")
                    nc.scalar.sign(sl[:], psum[:, 0:512],
                                   bias=b1_sb[:, m:m + 1])
                    nc.scalar.sign(a22r1[:, kk, :], psum[:, 512:BC],
                                   bias=b1_sb[:, m:m + 1])
                    s2_chunks(kk, sl)

        ps1.release()
        warmps.release()
        w1pool.release()
        xtq.release()
        stg1p.release()
        h1rp.release()

        # ---------------- right-stack pools: fc2 outputs ----------------
        # Live from here to the end of fc3; the left stack keeps cycling
        # its phase pools underneath.
        s3pool = tc.alloc_tile_pool(name="s3pool", bufs=1, side="right")
        h2lp = tc.alloc_tile_pool(name="h2lp", bufs=1, side="right")
        a22r2p = tc.alloc_tile_pool(name="a22r2p", bufs=1, side="right")
        stg2p = tc.alloc_tile_pool(name="stg2p", bufs=1, side="right")

        S3 = {}
        for px in range(2):
            for i in (1, 2, 5, 6, 7):
                S3[(i, px)] = s3pool.tile([128, NF, 256], dt.float8e4,
                                          tag=f"s3_{i}_{px}",
                                          name=f"s3_{i}_{px}")
        h2L = h2lp.tile([128, NF, 512], dt.float8e4)
        a22r2 = a22r2p.tile([128, NF, 512], dt.float8e4)

        # ------------- fc2 (one-level Strassen) -------------
        # lgp (fc4's logit accumulator) sits below psp on the PSUM stack so
        # psp can release before the tail needs its banks for transposes.
        # fc2+fc3 share the one product-psum pool (same tags) so fc3's
        # first matmul has no cross-pool PSUM overlap dependency.
        lgp = tc.alloc_tile_pool(name="lgp", bufs=1, space="PSUM")
        lg_psum = lgp.tile([16, BC], dt.float32)
        ps2 = tc.alloc_tile_pool(name="psp", bufs=1, space="PSUM")

        dmae = [nc.sync, nc.gpsimd]
        nd = [0]

        def s3_parts(f, px, stg21, stg12):
            """Per-op builders for the px-quarter of chunk f of the fc3
            S-tiles, emitted as soon as their sign inputs exist: h2L row f
            = A11 (i=1), stg21 = A21 (i=2), stg12 = A12 (i=3), a22r2 row f
            = A22 (i=6)."""
            c0 = px * 256
            L = h2L[:, f, c0:c0 + 256]
            A21 = stg21[:, c0:c0 + 256]
            A12 = stg12[:, c0:c0 + 256]
            A22 = a22r2[:, f, c0:c0 + 256]
            return {
                2: lambda: tt(S3[(6, px)][:, f, :], A21, L, SUB),
                3: lambda: tt(S3[(5, px)][:, f, :], L, A12, ADD),
                6: lambda: (tt(S3[(7, px)][:, f, :], A12, A22, SUB),
                            tt(S3[(1, px)][:, f, :], L, A22, ADD),
                            tt(S3[(2, px)][:, f, :], A21, A22, ADD)),
            }

        for f in range(NF):
            wts = {}
            if f == 0:
                wts.update(w2f0)
                for i in (6,):
                    t = w2pool.tile([128, KBS, 2, 128], dt.float8e4, tag="w2")
                    eng = dmae[nd[0] % 2]
                    nd[0] += 1
                    eng.dma_start(out=t[:], in_=w2s[0, :, i - 1])
                    wts[i] = t
            else:
                for i in ORDER:
                    t = w2pool.tile([128, KBS, 2, 128], dt.float8e4, tag="w2")
                    eng = dmae[nd[0] % 2]
                    nd[0] += 1
                    eng.dma_start(out=t[:], in_=w2s[f, :, i - 1])
                    wts[i] = t
                if f == NF - 1:
                    # fc3's f=0 weight tiles ride the (still-live) w2pool:
                    # their DMAs drain during this iteration's compute, so
                    # the fc2->fc3 switch never waits on weight DMA
                    for i in (5, 1, 2, 3, 6):
                        t = w2pool.tile([128, KBS, 2, 128], dt.float8e4,
                                        tag="w2")
                        eng = dmae[nd[0] % 2]
                        nd[0] += 1
                        eng.dma_start(out=t[:], in_=w3s[0, :, i - 1])
                        w3f0_pre[i] = t
            stg21 = stg2p.tile([128, 512], dt.float8e4,
                               tag=f"s21_{f % 2}", name="s21")
            stg12 = stg2p.tile([128, 512], dt.float8e4,
                               tag=f"s12_{f % 2}", name="s12")
            for px in range(2):
                c0 = px * 256
                s3b = s3_parts(f, px, stg21, stg12)

                def moving2(i, blk, c0=c0, px=px):
                    if i == 3:
                        return h1L[:, 2 * blk:2 * blk + 2, c0:c0 + 256]
                    if i == 4:
                        return a22r1[:, 2 * blk:2 * blk + 2, c0:c0 + 256]
                    return S2[(i, px)][:, 2 * blk:2 * blk + 2, :]
                # 7 quarter-size products packed pairwise into 2KB PSUM
                # banks; pairs chosen by lifetime so bufs=1 tags never
                # stall the PE
                pA = ps2.tile([128, 512], dt.float32, tag="pA", name="pA",
                              bufs=2)
                pB = ps2.tile([128, 512], dt.float32, tag="pB", name="pB")
                pC = ps2.tile([128, 512], dt.float32, tag="pC", name="pC")
                pD = ps2.tile([128, 256], dt.float32, tag="pD", name="pD",
                              bufs=2)
                psm = {4: pA[:, 0:256], 7: pA[:, 256:512],
                       5: pB[:, 0:256], 1: pB[:, 256:512],
                       2: pC[:, 0:256], 3: pC[:, 256:512],
                       6: pD[:, 0:256]}

                def ctile(tag, bufs=1):
                    return c2pool.tile([128, 256], dt.float32,
                                       tag=tag, name=tag, bufs=bufs)
                for i in ORDER:
                    for blk in range(KBS):
                        nc.tensor.matmul(
                            psm[i], wts[i][:, blk],
                            moving2(i, blk),
                            start=(blk == 0),
                            stop=(blk == KBS - 1),
                            perf_mode=DR,
                        )
                    if i == 4:
                        m4s = ctile("m4s")
                        nc.scalar.activation(m4s[:], psm[4], AF.Identity)
                    elif i == 7:
                        x1 = ctile("t_a")
                        tt(x1[:], m4s[:], psm[7], ADD)
                    elif i == 5:
                        m5s = ctile("m5s")
                        nc.scalar.activation(m5s[:], psm[5], AF.Identity)
                    elif i == 1:
                        x2 = ctile("t_b")
                        tt(x2[:], x1[:], psm[1], ADD)
                        c11 = ctile("t_d")
                        tt(c11[:], x2[:], m5s[:], SUB)
                        nc.scalar.sign(h2L[:, f, c0:c0 + 256], c11[:],
                                       bias=b2_sb[:, f:f + 1])
                    elif i == 2:
                        c21 = ctile("t_e")
                        tt(c21[:], m4s[:], psm[2], ADD)
                        nc.scalar.sign(stg21[:, c0:c0 + 256], c21[:],
                                       bias=b2_sb[:, f:f + 1])
                        m2s = ctile("m2s")
                        nc.scalar.activation(m2s[:], psm[2], AF.Identity)
                        s3b[2]()
                    elif i == 3:
                        c12 = ctile("t_d")
                        tt(c12[:], m5s[:], psm[3], ADD)
                        nc.scalar.sign(stg12[:, c0:c0 + 256], c12[:],
                                       bias=b2_sb[:, 24 + f:25 + f])
                        s3b[3]()
                        y2 = ctile("t_a")
                        nc.vector.scalar_tensor_tensor(
                            y2[:], m2s[:], -1.0, psm[1], MULT, ADD)
                        y3 = ctile("t_b")
                        tt(y3[:], y2[:], psm[3], ADD)
                    elif i == 6:
                        c22 = ctile("t_e")
                        tt(c22[:], y3[:], psm[6], ADD)
                        nc.scalar.sign(a22r2[:, f, c0:c0 + 256], c22[:],
                                       bias=b2_sb[:, 24 + f:25 + f])
                        s3b[6]()

        s2pool.release()
        a22r1p.release()
        h1lp.release()

        # ------------- fc3 (Strassen) + fused fc4 -------------
        if True:
            with tc.tile_pool(name="w3pool", bufs=8) as w3pool, \
                 tc.tile_pool(name="c3pool", bufs=1) as cp3, \
                 tc.tile_pool(name="h3pool", bufs=8) as h3pool:
                ps3 = ps2
                dmae3 = [nc.sync, nc.gpsimd]
                nd3 = [0]
                w3f0 = dict(w3pre)
                w3f0.update(w3f0_pre)

                h3_tiles = {}

                def fc4_pair(j, ns=(0, 1), stop=False):
                    t_h3 = h3_tiles[j]
                    for n in ns:
                        nc.tensor.matmul(
                            lg_psum[:, n * 512:(n + 1) * 512],
                            w4_sb[:, j],
                            t_h3[:, :, n * 512:(n + 1) * 512],
                            start=(j == 0),
                            stop=stop,
                            perf_mode=DR,
                        )

                def h3_store(m, cols, csb):
                    sl = h3_tiles[m % NF][:, m // NF, cols[0]:cols[1]]
                    nc.scalar.activation(sl, csb[:], AF.Identity,
                                         bias=b3_sb[:, m:m + 1])
                    nc.vector.tensor_scalar(sl, sl, 1.0, -1.0,
                                            mybir.AluOpType.min,
                                            mybir.AluOpType.max)

                # fc3 loops: f outer (so h3/fc4 drain promptly), px inner
                for f in range(NF):
                    h3_tiles[f] = h3pool.tile([128, 2, BC], dt.float8e4,
                                              tag="h3", name="h3")
                    wts = {}
                    if f == 0:
                        wts = w3f0
                    else:
                        for i in ORDER:
                            t = w3pool.tile([128, KBS, 2, 128],
                                            dt.float8e4, tag="w3")
                            eng = dmae3[nd3[0] % 2]
                            nd3[0] += 1
                            eng.dma_start(out=t[:], in_=w3s[f, :, i - 1])
                            wts[i] = t
                    for px in range(2):
                        c0 = px * 256

                        def moving3(i, blk, c0=c0, px=px):
                            if i == 3:
                                return h2L[:, 2 * blk:2 * blk + 2,
                                           c0:c0 + 256]
                            if i == 4:
                                return a22r2[:, 2 * blk:2 * blk + 2,
                                             c0:c0 + 256]
                            return S3[(i, px)][:, 2 * blk:2 * blk + 2, :]
                        qA = ps3.tile([128, 512], dt.float32,
                                      tag="pA", name="qA", bufs=2)
                        qB = ps3.tile([128, 512], dt.float32,
                                      tag="pB", name="qB")
                        qC = ps3.tile([128, 512], dt.float32,
                                      tag="pC", name="qC")
                        qD = ps3.tile([128, 256], dt.float32,
                                      tag="pD", name="qD", bufs=2)
                        psm = {4: qA[:, 0:256], 7: qA[:, 256:512],
                               5: qB[:, 0:256], 1: qB[:, 256:512],
                               2: qC[:, 0:256], 3: qC[:, 256:512],
                               6: qD[:, 0:256]}

                        def ctile3(tag, bufs=1):
                            return cp3.tile([128, 256], dt.float32,
                                            tag=tag, name=tag, bufs=bufs)
                        for i in ORDER:
                            for blk in range(KBS):
                                nc.tensor.matmul(
                                    psm[i], wts[i][:, blk],
                                    moving3(i, blk),
                                    start=(blk == 0),
                                    stop=(blk == KBS - 1),
                                    perf_mode=DR,
                                )
                            if i == 4:
                                m4s = ctile3("f3m4s")
                                nc.scalar.activation(m4s[:], psm[4],
                                                     AF.Identity)
                            elif i == 7:
                                x1 = ctile3("f3x1")
                                tt(x1[:], m4s[:], psm[7], ADD)
                            elif i == 5:
                                m5s = ctile3("f3m5s")
                                nc.scalar.activation(m5s[:], psm[5],
                                                     AF.Identity)
                            elif i == 1:
                                x2 = ctile3("f3x2")
                                tt(x2[:], x1[:], psm[1], ADD)
                                c11 = ctile3("f3c11", 2)
                                tt(c11[:], x2[:], m5s[:], SUB)
                                h3_store(f, (c0, c0 + 256), c11)
                            elif i == 2:
                                c21 = ctile3("f3c21", 2)
                                tt(c21[:], m4s[:], psm[2], ADD)
                                h3_store(f, (512 + c0, 512 + c0 + 256), c21)
                                m2s = ctile3("f3m2s")
                                nc.scalar.activation(m2s[:], psm[2],
                                                     AF.Identity)

                            elif i == 3:
                                c12 = ctile3("f3c12", 2)
                                tt(c12[:], m5s[:], psm[3], ADD)
                                h3_store(24 + f, (c0, c0 + 256), c12)
                                if f == NF - 1 and px == 1:
                                    fc4_pair(f, ns=(0,), stop=True)
                                y2 = ctile3("f3y2")
                                nc.vector.scalar_tensor_tensor(
                                    y2[:], m2s[:], -1.0, psm[1],
                                    MULT, ADD)
                                y3 = ctile3("f3y3")
                                tt(y3[:], y2[:], psm[3], ADD)
                            elif i == 6:
                                c22 = ctile3("f3c22", 2)
                                tt(c22[:], y3[:], psm[6], ADD)
                                h3_store(24 + f,
                                         (512 + c0, 512 + c0 + 256), c22)
                                if f == NF - 1 and px == 1:
                                    fc4_pair(f, ns=(1,), stop=True)
                    # fc4, pipelined one f behind
                    if f > 0:
                        fc4_pair(f - 1)
                    if f == NF - 2:
                        # pull the Exp table into the ACT engine well ahead
                        # of the tail (a function switch costs a 1.28us
                        # ACT_TABLE_LOAD; the last f's ACT queue is too
                        # backlogged to absorb it)
                        nc.scalar.activation(warm[:], ident[0:1, 0:1],
                                             AF.Exp)
            ps2.release()

            # ------------- bias + log_softmax (max-free) -------------
            # logits are bounded (|l| < 40), so exp() cannot overflow fp32
            # and the rowmax subtraction is unnecessary:
            # out = l - ln(sum(exp(l))). Partition 32 of lg_sb holds the
            # per-column ln-sum so one PE transpose carries both. The Exp
            # activation table was pre-warmed during the last fc4 matmuls,
            # so only the Ln switch pays a table load here.
            NJ = BC // 128
            with tc.tile_pool(name="tp", bufs=1, space="PSUM") as tpp, \
                 tc.tile_pool(name="sm", bufs=1) as smp:
                lg_sb = smp.tile([33, BC], dt.float32)
                ex_sb = smp.tile([DOUT, BC], dt.float16, tag="ex")
                sums_ps = tpp.tile([1, BC], dt.float32, tag="sums")
                # halved exp -> sums -> ln chain so each stage starts as
                # soon as half its input exists
                for n in range(NH):
                    nc.scalar.activation(ex_sb[:, n * 512:(n + 1) * 512],
                                         lg_psum[0:DOUT,
                                                 n * 512:(n + 1) * 512],
                                         AF.Exp, bias=b4_sb[:, 0:1],
                                         scale=1.0 / 64.0)
                    if n == 0:
                        # pull the Ln table in behind the Exps so the real
                        # Ln doesn't pay the 1.28us load serially
                        nc.scalar.activation(warm[:], warm[:], AF.Ln)
                # biased logits copy runs on the DVE, in parallel with the
                # ACT engine's Exp
                nc.vector.tensor_scalar(lg_sb[0:DOUT, :],
                                        lg_psum[0:DOUT, :], 1.0 / 64.0,
                                        b4_sb[:, 0:1], MULT, ADD)
                for n in range(NH):
                    nc.tensor.matmul(
                        sums_ps[:, n * 512:(n + 1) * 512],
                        ones_sb[:, 0:1],
                        ex_sb[:, n * 512:(n + 1) * 512],
                    )
                    nc.scalar.activation(
                        lg_sb[32:33, n * 512:(n + 1) * 512],
                        sums_ps[:, n * 512:(n + 1) * 512], AF.Ln)
                dmo = [nc.sync, nc.sync]
                for j in range(NJ):
                    tp = tpp.tile([128, 33], dt.float32, tag=f"tp{j % 2}")
                    nc.tensor.transpose(
                        tp[:], lg_sb[:, j * 128:(j + 1) * 128], ident[:])
                    res = smp.tile([128, DOUT], dt.float32, tag=f"res{j}")
                    nc.vector.tensor_scalar(res[:], tp[:, 0:DOUT],
                                            tp[:, 32:33], None, SUB)
                    dmo[j % 2].dma_start(
                        out=out[j * 128:(j + 1) * 128, :], in_=res[:])

        # releases (reverse open order per side)
        lgp.release()
        c2pool.release()
        w2pool.release()
        stg2p.release()
        a22r2p.release()
        h2lp.release()
        s3pool.release()
        cpool.release()

    nc.compile()
    return nc


def _pack_inputs(x, w1, b1, w2, b2, w3, b3, w4, b4):
    """Host-side packing into the device layouts. Shared tensors are packed
    once; only xt differs per core."""
    f32 = np.float32
    f16 = np.float16
    x = np.asarray(x, f32).reshape(B, DIN)

    # fc1 weights: sign(w1).T stacked twice (hi/lo terms share the weights),
    # padded to [1664, 6144], layout [q, p, k, m]
    s1 = np.sign(np.asarray(w1, f32))                       # [DH, DIN]
    s1t = np.zeros((K1P, DH), f16)
    s1t[:DIN] = s1.T
    s1t[DIN:2 * DIN] = s1.T
    w1t = np.ascontiguousarray(
        s1t.reshape(KT1, 128, MQ, MPQ * 128).transpose(2, 1, 0, 3))

    # fc2/fc3 weights: Strassen T-combos of sign(w).T, DoubleRow layout per
    # 128-wide output chunk: [fo, p, 7, blk, i2, f']
    def pack_strassen(w):
        sm = np.sign(np.asarray(w, f32)).T                  # [in, out] = B
        H = DH // 2
        B11 = sm[:H, :H]
        B12 = sm[:H, H:]
        B21 = sm[H:, :H]
        B22 = sm[H:, H:]
        Ts = [B11 + B22, B11, B12 - B22, B21 - B11, B22, B11 + B12,
              B21 + B22]

        def pack_t(t):   # [3072, 3072] -> [fo, p, blk, i2, f']
            r = t.reshape(KBS, 2, 128, NF, 128)
            return r.transpose(3, 2, 0, 1, 4)

        return np.ascontiguousarray(
            np.stack([pack_t(t) for t in Ts], axis=2)).astype(FP8)

    w2sp = pack_strassen(w2)
    w3sp = pack_strassen(w3)

    # fc4 weights: w4.T scaled by 64 into fp8 (subnormal-free), DoubleRow
    # pair layout [p, j, t, c'] with pair (j, 24+j); the 1/64 is undone in
    # the log_softmax tail's scale factor
    w4t = np.asarray(w4, f32).T * 64.0                      # [DH, DOUT]
    w4r = w4t.reshape(MT, 128, DOUT)
    w4pk = np.zeros((128, NF, 2, 16), f32)
    for t in range(2):
        w4pk[:, :, t, :DOUT] = w4r[24 * t:24 * t + 24].transpose(1, 0, 2)
    w4pk = np.ascontiguousarray(w4pk).astype(FP8)

    def pack_b(b):
        return np.ascontiguousarray(np.asarray(b, f32).reshape(MT, 128).T)

    b1pk, b2pk, b3pk = pack_b(b1), pack_b(b2), pack_b(b3)
    b4pk = np.asarray(b4, f32).reshape(DOUT, 1)

    shared = {"w1t": w1t, "w2s": w2sp, "w3s": w3sp, "w4p": w4pk,
              "b1p": b1pk, "b2p": b2pk, "b3p": b3pk, "b4p": b4pk}

    # per-core x: fp16 hi/lo split stacked along contraction, layout [p, k, n]
    in_maps = []
    for c in range(CORES):
        xc = x[c * BC:(c + 1) * BC]                         # [BC, DIN]
        hi = xc.astype(f16)
        lo = (xc - hi.astype(f32)).astype(f16)
        arr = np.zeros((K1P, BC), f16)
        arr[:DIN] = hi.T
        arr[DIN:2 * DIN] = lo.T
        xtc = np.ascontiguousarray(arr.reshape(KT1, 128, BC).transpose(1, 0, 2))
        in_maps.append({"xt": xtc, **shared})
    return in_maps


_cached_nc = None


def kernel(x, w1, b1, w2, b2, w3, b3, w4, b4):
    global _cached_nc, last_exec_time_ns
    import os
    trace = bool(int(os.environ.get("KERNEL_TRACE", "0")))
    if _cached_nc is None:
        _cached_nc = _build_program()
    in_maps = _pack_inputs(x, w1, b1, w2, b2, w3, b3, w4, b4)
    res = run_bass_kernel_spmd(_cached_nc, in_maps, list(range(CORES)),
                               trace=trace)
    last_exec_time_ns = res.exec_time_ns
    return np.concatenate([res.results[c]["out"] for c in range(CORES)], axis=0)
